# revision 29
# baseline (speedup 1.0000x reference)
"""Trainium2 Bass kernel for LocalSemanticAlignment (sparse_attention).

Pipeline (reference semantics):
  masks   = parse[:,1:] downsampled 256->64 (nearest, stride-4)
  ufb     = bilinear-AC downsample of unalign_fb to 64x64        (host)
  fan/fbn = per-channel-centered, per-column L2-normalized fa/fb (host)
  S[q,p]  = fbn^T fan                                            (device, fp32r matmul)
  per class k: w_k = where(mask_b[q], exp(alpha*S), 1) masked softmax over q
  warped_k = ufb @ softmax  ->  combined over k with mask_a / counts
  output  = bilinear-AC upsample of aligned to 256x256           (host)

Key identity used on device: w_k = mask_b[q]*exp(alpha*S) + (1-mask_b[q]), so
  numer_k = (ufb*mask_b[k])^T @ E + const_k,  denom_k = mask_b[k]^T @ E + z_k
with E = exp(alpha*S) shared across classes. All masking is folded into the
stationary (lhsT) operands, so the device loop is pure matmul + Exp.

Sharding: the 4096 output columns (p axis) are split 512 per core; every core
holds full fbn/ufb (keys/values) and computes its shard end-to-end. No
collectives.
"""

import numpy as np
import ml_dtypes

import concourse.bass as bass
import concourse.bacc as bacc
import concourse.mybir as mybir
from concourse import tile
from concourse.bass_utils import run_bass_kernel_spmd

ALPHA = 100.0
# global logit shift: exp(alpha*S - CSHIFT) everywhere, with the "+1" weights
# of masked-out keys scaled by exp(-CSHIFT) on the host (vks/zk). Softmax is
# shift-invariant so this is exact; it keeps exp() in f32/bf16 range for
# logits up to CSHIFT+88 (observed max ~90).
CSHIFT = 60.0
N_CORES = 8
HW = 4096          # 64*64 spatial positions at feature resolution
PCOLS = HW // N_CORES   # p columns per core = 512
NQT = HW // 128    # 32 q-tiles of 128
UC = 195           # ucomb cols per q-tile: U0|U1|U2|mb0|mb1|mb2 = 64*3+3

F32 = mybir.dt.float32
F32R = mybir.dt.float32r
BF16 = mybir.dt.bfloat16


def _interp_bilinear_ac(x, size):
    """torch F.interpolate bilinear align_corners=True; x: (C,H,W) float32."""
    x = np.ascontiguousarray(x, np.float32)
    H, W = x.shape[-2], x.shape[-1]
    h, w = size

    def coords(n_out, n_in):
        if n_out == 1:
            return np.zeros((1,), np.float32)
        return np.arange(n_out, dtype=np.float32) * np.float32((n_in - 1) / (n_out - 1))

    ry, rx = coords(h, H), coords(w, W)
    y0 = np.floor(ry).astype(np.int32)
    x0 = np.floor(rx).astype(np.int32)
    y1 = np.clip(y0 + 1, 0, H - 1)
    x1 = np.clip(x0 + 1, 0, W - 1)
    wy = (ry - y0.astype(np.float32))[None, :, None]
    wx = (rx - x0.astype(np.float32))[None, None, :]
    rows = x[:, y0, :] * (1.0 - wy) + x[:, y1, :] * wy
    return (rows[:, :, x0] * (1.0 - wx) + rows[:, :, x1] * wx).astype(np.float32)


_NC_CACHE = {}


def _build_program():
    if "nc" in _NC_CACHE:
        return _NC_CACHE["nc"]

    nc = bacc.Bacc("TRN2", target_bir_lowering=False, debug=False,
                   num_devices=N_CORES)

    fbn_d = nc.dram_tensor("fbn", [2, 128, HW], F32R, kind="ExternalInput").ap()
    fan_d = nc.dram_tensor("fan", [2, 128, PCOLS], F32R, kind="ExternalInput").ap()
    uc_d = nc.dram_tensor("ucomb", [128, NQT * UC], BF16, kind="ExternalInput").ap()
    ga_d = nc.dram_tensor("ga", [3, PCOLS], F32, kind="ExternalInput").ap()
    vz_d = nc.dram_tensor("vz", [128, 4], F32, kind="ExternalInput").ap()
    zk_d = nc.dram_tensor("zk", [3, 1], F32, kind="ExternalInput").ap()
    e3_d = nc.dram_tensor("e3", [3, 192], F32R, kind="ExternalInput").ap()
    out_d = nc.dram_tensor("out_al", [64, PCOLS], F32, kind="ExternalOutput").ap()

    EXP = mybir.ActivationFunctionType.Exp
    CHUNK = 4  # q-tiles per DMA chunk for fbn/ucomb streaming

    with tile.TileContext(nc) as tc:
        with (
            tc.tile_pool(name="io", bufs=1) as io,
            tc.tile_pool(name="big", bufs=1) as big,
            tc.tile_pool(name="expp", bufs=3) as expp,
            tc.tile_pool(name="spsum", bufs=2, space="PSUM") as spsum,
            tc.tile_pool(name="npsum", bufs=1, space="PSUM") as npsum,
            tc.tile_pool(name="gpsum", bufs=2, space="PSUM") as gpsum,
            tc.tile_pool(name="fin", bufs=1) as fin,
        ):
            fan_sb = []
            for c in range(2):
                t_ = io.tile([128, PCOLS], F32R, tag=f"fan{c}", name=f"fan_sb{c}")
                nc.sync.dma_start(t_[:], fan_d[c])
                fan_sb.append(t_)

            ga_sb = io.tile([3, PCOLS], F32, tag="ga")
            nc.sync.dma_start(ga_sb[:], ga_d[:])
            vz_sb = io.tile([128, 4], F32, tag="vz")
            nc.sync.dma_start(vz_sb[:], vz_d[:])
            zk_sb = io.tile([3, 1], F32, tag="zk")
            nc.sync.dma_start(zk_sb[:], zk_d[:])
            e3_sb = io.tile([3, 192], F32R, tag="e3")
            nc.sync.dma_start(e3_sb[:], e3_d[:])
            cb_sb = io.tile([128, 1], F32, tag="cb")
            nc.vector.memset(cb_sb[:], -CSHIFT)

            fbn_sb = [big.tile([128, HW], F32R, tag=f"fbn{c}", name=f"fbn_sb{c}")
                      for c in range(2)]
            uc_sb = big.tile([128, NQT * UC], BF16, tag="ucomb")
            # stream loads in q order so compute can start after chunk 0
            for ch in range(NQT // CHUNK):
                qs = slice(ch * CHUNK * 128, (ch + 1) * CHUNK * 128)
                us = slice(ch * CHUNK * UC, (ch + 1) * CHUNK * UC)
                for c in range(2):
                    nc.sync.dma_start(fbn_sb[c][:, qs], fbn_d[c][:, qs])
                nc.scalar.dma_start(uc_sb[:, us], uc_d[:, us])

            n1_ps = npsum.tile([128, PCOLS], F32, tag="n1")
            n2_ps = npsum.tile([67, PCOLS], F32, tag="n2")

            for t in range(NQT):
                qs = slice(t * 128, (t + 1) * 128)
                s_ps = spsum.tile([128, PCOLS], F32, tag="s")
                nc.tensor.matmul(s_ps[:], fbn_sb[0][:, qs], fan_sb[0][:],
                                 start=True, stop=False)
                nc.tensor.matmul(s_ps[:], fbn_sb[1][:, qs], fan_sb[1][:],
                                 start=False, stop=True)
                e_sb = expp.tile([128, PCOLS], BF16, tag="e")
                nc.scalar.activation(e_sb[:], s_ps[:], EXP, scale=ALPHA,
                                     bias=cb_sb[:])
                u0 = t * UC
                nc.tensor.matmul(n1_ps[:], uc_sb[:, u0:u0 + 128], e_sb[:],
                                 start=(t == 0), stop=(t == NQT - 1))
                nc.tensor.matmul(n2_ps[:], uc_sb[:, u0 + 128:u0 + UC], e_sb[:],
                                 start=(t == 0), stop=(t == NQT - 1))

            # epilogue: out[c,p] = sum_k g_k[p] * (numer_k[c,p] + v_k[c])
            # with g_k = mask_a[k]/norm / (denom_k + z_k)
            d_sb = fin.tile([3, PCOLS], F32, tag="d")
            g_sb = fin.tile([3, PCOLS], F32R, tag="g")
            acc_sb = fin.tile([64, PCOLS], F32, tag="acc")
            t1_sb = fin.tile([64, PCOLS], F32, tag="t1")
            t2_sb = fin.tile([64, PCOLS], F32, tag="t2")
            nc.vector.tensor_scalar_add(d_sb[:], n2_ps[64:67, :],
                                        zk_sb[0:3, 0:1])
            nc.vector.reciprocal(d_sb[:], d_sb[:])
            nc.vector.tensor_mul(g_sb[:], d_sb[:], ga_sb[:])
            # broadcast g_k rows across 64 partitions via one-hot matmuls:
            # gb1[0:64]=g0, gb1[64:128]=g1, gb2[0:64]=g2
            gb1_ps = gpsum.tile([128, PCOLS], F32, tag="gb1")
            gb2_ps = gpsum.tile([64, PCOLS], F32, tag="gb2")
            nc.tensor.matmul(gb1_ps[:], e3_sb[:, 0:128], g_sb[:],
                             start=True, stop=True)
            nc.tensor.matmul(gb2_ps[:], e3_sb[:, 128:192], g_sb[:],
                             start=True, stop=True)
            for k in range(3):
                if k < 2:
                    numer = n1_ps[64 * k:64 * k + 64, :]
                    vzk = vz_sb[64 * k:64 * k + 64, k:k + 1]
                    gbk = gb1_ps[64 * k:64 * k + 64, :]
                else:
                    numer = n2_ps[0:64, :]
                    vzk = vz_sb[0:64, k:k + 1]
                    gbk = gb2_ps[0:64, :]
                nc.vector.tensor_scalar_add(t1_sb[:], numer, vzk)
                if k == 0:
                    nc.vector.tensor_mul(acc_sb[:], t1_sb[:], gbk)
                else:
                    nc.vector.tensor_mul(t2_sb[:], t1_sb[:], gbk)
                    nc.vector.tensor_add(acc_sb[:], acc_sb[:], t2_sb[:])

            nc.sync.dma_start(out_d[:], acc_sb[:])

    nc.compile()
    _NC_CACHE["nc"] = nc
    return nc


def _prep_inputs(unalign_fb, fa, fa_parse, fb, fb_parse):
    c2 = unalign_fb.shape[1]
    c = fa.shape[1]
    mask_a = (fa_parse[0, 1:, ::4, ::4].reshape(3, HW) != 0).astype(np.float32)
    mask_b = (fb_parse[0, 1:, ::4, ::4].reshape(3, HW) != 0).astype(np.float32)
    ufb = _interp_bilinear_ac(unalign_fb[0], (64, 64)).reshape(c2, HW)

    faf = np.ascontiguousarray(fa[0].reshape(c, HW), np.float32)
    fbf = np.ascontiguousarray(fb[0].reshape(c, HW), np.float32)
    faf = faf - faf.mean(axis=1, keepdims=True, dtype=np.float32)
    fbf = fbf - fbf.mean(axis=1, keepdims=True, dtype=np.float32)
    fan = faf / np.linalg.norm(faf, axis=0, keepdims=True)
    fbn = fbf / np.linalg.norm(fbf, axis=0, keepdims=True)

    # stationary operands for the numerator/denominator matmuls, tiled per
    # 128-q block: [U0|U1|U2|mb0|mb1|mb2] transposed to [q,cols]
    U = ufb[None] * mask_b[:, None, :]                     # (3,64,HW)
    ucomb = np.empty((128, NQT * UC), np.float32)
    Ut = U.transpose(2, 0, 1).reshape(HW, 3 * 64)          # (HW, 192) q-major
    mbt = mask_b.T                                         # (HW, 3)
    for t in range(NQT):
        qs = slice(t * 128, (t + 1) * 128)
        ucomb[:, t * UC:t * UC + 192] = Ut[qs]
        ucomb[:, t * UC + 192:t * UC + 195] = mbt[qs]
    ucomb = ucomb.astype(ml_dtypes.bfloat16)

    norm = np.maximum(mask_a.sum(axis=0), 1.0)
    ga = (mask_a / norm[None, :]).astype(np.float32)       # (3,HW)
    esc = np.float32(np.exp(-CSHIFT))
    vks = (ufb @ (1.0 - mask_b).T).astype(np.float32) * esc  # (64,3)
    vz = np.zeros((128, 4), np.float32)
    vz[0:64, 0:3] = vks
    vz[64:128, 0:3] = vks
    zk = np.ascontiguousarray((1.0 - mask_b).sum(axis=1, dtype=np.float32)
                              [:, None] * esc)
    e3 = np.zeros((3, 192), np.float32)
    e3[0, 0:64] = 1.0
    e3[1, 64:128] = 1.0
    e3[2, 128:192] = 1.0

    fbn3 = np.ascontiguousarray(fbn.reshape(2, 128, HW))
    fan3 = np.ascontiguousarray(fan.reshape(2, 128, HW))
    in_maps = []
    for i in range(N_CORES):
        ps = slice(i * PCOLS, (i + 1) * PCOLS)
        in_maps.append({
            "fbn": fbn3,
            "fan": np.ascontiguousarray(fan3[:, :, ps]),
            "ucomb": ucomb,
            "ga": np.ascontiguousarray(ga[:, ps]),
            "vz": vz,
            "zk": zk,
            "e3": e3,
        })
    return in_maps


def _run(inputs, trace=False, trace_cores=None):
    unalign_fb = np.asarray(inputs["unalign_fb"], np.float32)
    fa = np.asarray(inputs["fa"], np.float32)
    fa_parse = np.asarray(inputs["fa_parse"])
    fb = np.asarray(inputs["fb"], np.float32)
    fb_parse = np.asarray(inputs["fb_parse"])

    nc = _build_program()
    in_maps = _prep_inputs(unalign_fb, fa, fa_parse, fb, fb_parse)
    res = run_bass_kernel_spmd(nc, in_maps, core_ids=list(range(N_CORES)),
                               trace=trace, trace_cores=trace_cores)

    c2 = unalign_fb.shape[1]
    aligned = np.empty((c2, HW), np.float32)
    for i in range(N_CORES):
        aligned[:, i * PCOLS:(i + 1) * PCOLS] = res.results[i]["out_al"]
    out = _interp_bilinear_ac(aligned.reshape(c2, 64, 64), (256, 256))
    return out[None], res


def kernel(**inputs):
    out, _ = _run(inputs)
    return out


# revision 30
# speedup vs baseline: 1.4683x; 1.4683x over previous
"""Trainium2 Bass kernel for LocalSemanticAlignment (sparse_attention).

Pipeline (reference semantics):
  masks   = parse[:,1:] downsampled 256->64 (nearest, stride-4)
  ufb     = bilinear-AC downsample of unalign_fb to 64x64        (host)
  fan/fbn = per-channel-centered, per-column L2-normalized fa/fb (host)
  S[q,p]  = fbn^T fan                                            (device, fp32r matmul)
  per class k: w_k = where(mask_b[q], exp(alpha*S - C), exp(-C)) masked
  softmax over q (shift C is exact; see CSHIFT)
  warped_k = ufb @ softmax  ->  combined over k with mask_a / counts
  output  = bilinear-AC upsample of aligned to 256x256           (host)

Key identity used on device: w_k = mask_b[q]*exp(aS-C) + (1-mask_b[q])e^-C, so
  numer_k = (ufb*mask_b[k])^T @ E + const_k,  denom_k = mask_b[k]^T @ E + z_k
with E = exp(aS - C) shared across classes. All masking is folded into the
stationary (lhsT) operands, so the device loop is pure matmul + Exp. The
numerators/denominators (195 x 512 per core) are shipped back and the final
divide+combine (trivial) happens on the host.

Sharding: the 4096 output columns (p axis) are split 512 per core; every core
holds full fbn/ufb (keys/values) and computes its shard end-to-end. No
collectives.
"""

import numpy as np
import ml_dtypes

import concourse.bass as bass
import concourse.bacc as bacc
import concourse.mybir as mybir
from concourse import tile
from concourse.bass_utils import run_bass_kernel_spmd

ALPHA = 100.0
# global logit shift: exp(alpha*S - CSHIFT) everywhere, with the "+1" weights
# of masked-out keys scaled by exp(-CSHIFT) on the host (vks/zk). Softmax is
# shift-invariant so this is exact; it keeps exp() in f32/bf16 range for
# logits up to CSHIFT+88 (observed max ~90).
CSHIFT = 60.0
N_CORES = 8
HW = 4096          # 64*64 spatial positions at feature resolution
PCOLS = HW // N_CORES   # p columns per core = 512
NQT = HW // 128    # 32 q-tiles of 128
UC = 195           # ucomb cols per q-tile: U0|U1|U2|mb0|mb1|mb2 = 64*3+3

F32 = mybir.dt.float32
F32R = mybir.dt.float32r
BF16 = mybir.dt.bfloat16


def _interp_bilinear_ac(x, size):
    """torch F.interpolate bilinear align_corners=True; x: (C,H,W) float32."""
    x = np.ascontiguousarray(x, np.float32)
    H, W = x.shape[-2], x.shape[-1]
    h, w = size

    def coords(n_out, n_in):
        if n_out == 1:
            return np.zeros((1,), np.float32)
        return np.arange(n_out, dtype=np.float32) * np.float32((n_in - 1) / (n_out - 1))

    ry, rx = coords(h, H), coords(w, W)
    y0 = np.floor(ry).astype(np.int32)
    x0 = np.floor(rx).astype(np.int32)
    y1 = np.clip(y0 + 1, 0, H - 1)
    x1 = np.clip(x0 + 1, 0, W - 1)
    wy = (ry - y0.astype(np.float32))[None, :, None]
    wx = (rx - x0.astype(np.float32))[None, None, :]
    rows = x[:, y0, :] * (1.0 - wy) + x[:, y1, :] * wy
    return (rows[:, :, x0] * (1.0 - wx) + rows[:, :, x1] * wx).astype(np.float32)


_NC_CACHE = {}


def _build_program():
    if "nc" in _NC_CACHE:
        return _NC_CACHE["nc"]

    nc = bacc.Bacc("TRN2", target_bir_lowering=False, debug=False,
                   num_devices=N_CORES)

    fbn_d = nc.dram_tensor("fbn", [2, 128, HW], F32R, kind="ExternalInput").ap()
    fan_d = nc.dram_tensor("fan", [2, 128, PCOLS], F32R, kind="ExternalInput").ap()
    uc_d = nc.dram_tensor("ucomb", [128, NQT * UC], BF16, kind="ExternalInput").ap()
    out_d = nc.dram_tensor("out_nd", [UC, PCOLS], F32, kind="ExternalOutput").ap()

    EXP = mybir.ActivationFunctionType.Exp
    CHUNK = 4    # q-tiles per DMA chunk for fbn/ucomb streaming
    NWARM = 16   # HAM warm-up matmuls issued while DMAs stream in

    with tile.TileContext(nc) as tc:
        with (
            tc.tile_pool(name="io", bufs=1) as io,
            tc.tile_pool(name="big", bufs=1) as big,
            tc.tile_pool(name="expp", bufs=4) as expp,
            tc.tile_pool(name="spsum", bufs=3, space="PSUM") as spsum,
            tc.tile_pool(name="npsum", bufs=1, space="PSUM") as npsum,
            tc.tile_pool(name="wpsum", bufs=1, space="PSUM") as wpsum,
            tc.tile_pool(name="fin", bufs=1) as fin,
        ):
            # PE warm-up: ~7us of dummy matmuls with no DMA deps keeps the
            # HAM activity window busy so real matmuls run at 2.4 GHz.
            wz_sb = io.tile([1, PCOLS], BF16, tag="wz")
            nc.vector.memset(wz_sb[:], 0.0)
            w_ps = wpsum.tile([1, PCOLS], F32, tag="wps")
            for _ in range(NWARM):
                nc.tensor.matmul(w_ps[:], wz_sb[0:1, 0:1], wz_sb[:],
                                 start=True, stop=True)

            fan_sb = []
            for c in range(2):
                t_ = io.tile([128, PCOLS], F32R, tag=f"fan{c}", name=f"fan_sb{c}")
                nc.sync.dma_start(t_[:], fan_d[c])
                fan_sb.append(t_)

            cb_sb = io.tile([128, 1], F32, tag="cb")
            nc.vector.memset(cb_sb[:], -CSHIFT)

            fbn_sb = [big.tile([128, HW], F32R, tag=f"fbn{c}", name=f"fbn_sb{c}")
                      for c in range(2)]
            uc_sb = big.tile([128, NQT * UC], BF16, tag="ucomb")
            # stream loads in q order so compute can start after chunk 0;
            # split issue across the two HWDGE engines + SWDGE so no single
            # queue serializes the 6 MB load.
            for ch in range(NQT // CHUNK):
                qs = slice(ch * CHUNK * 128, (ch + 1) * CHUNK * 128)
                us = slice(ch * CHUNK * UC, (ch + 1) * CHUNK * UC)
                nc.sync.dma_start(fbn_sb[0][:, qs], fbn_d[0][:, qs])
                nc.gpsimd.dma_start(fbn_sb[1][:, qs], fbn_d[1][:, qs])
                nc.sync.dma_start(uc_sb[:, us], uc_d[:, us])

            n1_ps = npsum.tile([128, PCOLS], F32, tag="n1")
            n2_ps = npsum.tile([67, PCOLS], F32, tag="n2")

            for t in range(NQT):
                qs = slice(t * 128, (t + 1) * 128)
                s_ps = spsum.tile([128, PCOLS], F32, tag="s")
                nc.tensor.matmul(s_ps[:], fbn_sb[0][:, qs], fan_sb[0][:],
                                 start=True, stop=False)
                nc.tensor.matmul(s_ps[:], fbn_sb[1][:, qs], fan_sb[1][:],
                                 start=False, stop=True)
                e_sb = expp.tile([128, PCOLS], BF16, tag="e")
                nc.scalar.activation(e_sb[:], s_ps[:], EXP, scale=ALPHA,
                                     bias=cb_sb[:])
                u0 = t * UC
                nc.tensor.matmul(n1_ps[:], uc_sb[:, u0:u0 + 128], e_sb[:],
                                 start=(t == 0), stop=(t == NQT - 1))
                nc.tensor.matmul(n2_ps[:], uc_sb[:, u0 + 128:u0 + UC], e_sb[:],
                                 start=(t == 0), stop=(t == NQT - 1))

            # ship raw numerators (rows 0:192) + denominators (rows 192:195)
            # to the host; the final divide+combine is trivial there.
            o1_sb = fin.tile([128, PCOLS], F32, tag="o1")
            o2_sb = fin.tile([67, PCOLS], F32, tag="o2")
            nc.vector.tensor_copy(o1_sb[:], n1_ps[:])
            nc.vector.tensor_copy(o2_sb[:], n2_ps[:])
            nc.sync.dma_start(out_d[0:128], o1_sb[:])
            nc.sync.dma_start(out_d[128:UC], o2_sb[:])

    nc.compile()
    _NC_CACHE["nc"] = nc
    return nc


def _prep_inputs(unalign_fb, fa, fa_parse, fb, fb_parse):
    c2 = unalign_fb.shape[1]
    c = fa.shape[1]
    mask_a = (fa_parse[0, 1:, ::4, ::4].reshape(3, HW) != 0).astype(np.float32)
    mask_b = (fb_parse[0, 1:, ::4, ::4].reshape(3, HW) != 0).astype(np.float32)
    ufb = _interp_bilinear_ac(unalign_fb[0], (64, 64)).reshape(c2, HW)

    faf = np.ascontiguousarray(fa[0].reshape(c, HW), np.float32)
    fbf = np.ascontiguousarray(fb[0].reshape(c, HW), np.float32)
    faf = faf - faf.mean(axis=1, keepdims=True, dtype=np.float32)
    fbf = fbf - fbf.mean(axis=1, keepdims=True, dtype=np.float32)
    fan = faf / np.linalg.norm(faf, axis=0, keepdims=True)
    fbn = fbf / np.linalg.norm(fbf, axis=0, keepdims=True)

    # stationary operands for the numerator/denominator matmuls, tiled per
    # 128-q block: [U0|U1|U2|mb0|mb1|mb2] transposed to [q,cols]
    U = ufb[None] * mask_b[:, None, :]                     # (3,64,HW)
    ucomb = np.empty((128, NQT * UC), np.float32)
    Ut = U.transpose(2, 0, 1).reshape(HW, 3 * 64)          # (HW, 192) q-major
    mbt = mask_b.T                                         # (HW, 3)
    for t in range(NQT):
        qs = slice(t * 128, (t + 1) * 128)
        ucomb[:, t * UC:t * UC + 192] = Ut[qs]
        ucomb[:, t * UC + 192:t * UC + 195] = mbt[qs]
    ucomb = ucomb.astype(ml_dtypes.bfloat16)

    fbn3 = np.ascontiguousarray(fbn.reshape(2, 128, HW))
    fan3 = np.ascontiguousarray(fan.reshape(2, 128, HW))
    in_maps = []
    for i in range(N_CORES):
        ps = slice(i * PCOLS, (i + 1) * PCOLS)
        in_maps.append({
            "fbn": fbn3,
            "fan": np.ascontiguousarray(fan3[:, :, ps]),
            "ucomb": ucomb,
        })

    # host-epilogue constants
    esc = np.float32(np.exp(-CSHIFT))
    norm = np.maximum(mask_a.sum(axis=0), 1.0)
    ga = (mask_a / norm[None, :]).astype(np.float32)            # (3,HW)
    vks = (ufb @ (1.0 - mask_b).T).astype(np.float32) * esc     # (64,3)
    zk = ((1.0 - mask_b).sum(axis=1).astype(np.float32) * esc)  # (3,)
    return in_maps, (ga, vks, zk)


def _run(inputs, trace=False, trace_cores=None):
    unalign_fb = np.asarray(inputs["unalign_fb"], np.float32)
    fa = np.asarray(inputs["fa"], np.float32)
    fa_parse = np.asarray(inputs["fa_parse"])
    fb = np.asarray(inputs["fb"], np.float32)
    fb_parse = np.asarray(inputs["fb_parse"])

    nc = _build_program()
    in_maps, (ga, vks, zk) = _prep_inputs(unalign_fb, fa, fa_parse, fb,
                                          fb_parse)
    res = run_bass_kernel_spmd(nc, in_maps, core_ids=list(range(N_CORES)),
                               trace=trace, trace_cores=trace_cores)

    c2 = unalign_fb.shape[1]
    aligned = np.zeros((c2, HW), np.float32)
    for i in range(N_CORES):
        nd = res.results[i]["out_nd"]                  # (195, 512)
        ps = slice(i * PCOLS, (i + 1) * PCOLS)
        for k in range(3):
            numer = nd[64 * k:64 * k + 64] + vks[:, k:k + 1]
            denom = nd[192 + k] + zk[k]
            aligned[:, ps] += (ga[k, ps] / denom)[None, :] * numer
    out = _interp_bilinear_ac(aligned.reshape(c2, 64, 64), (256, 256))
    return out[None], res


def kernel(**inputs):
    out, _ = _run(inputs)
    return out
